# revision 58
# baseline (speedup 1.0000x reference)
"""Raw-bass (manual sync) Trainium2 kernel for nn_MultiHeadAttention_79577154060910.

Math (verified vs the jax reference to ~2e-7 rel in fp32, ~3e-3 in bf16): the
reference's GLOBAL softmax (no axis) plus its sign-bugged causal mask
(`S - (1-tril)*(-1e9)` ADDS +1e9 to the strict upper triangle) make the second
softmax's weights an input-independent constant in fp32 arithmetic: every
strictly-upper-triangular position holds exactly 1/M (M = B*H*S*(S-1)/2 =
67076096) and all other positions are exactly exp(-1e9) == 0.  Hence q, k, WQ,
WK never affect the output and

    out[b, q, h*64+d] = (1/M) * sum_{k>q} V[b,h,k,d],  V = (v@WV).reshape(B,H,S,64)

With the raw-reshape head split (V[b,h,k,d] = VV[b, h*128+k//16, (k%16)*64+d]),
each (b,h) maps to a 128-row block of VV and, splitting k = 16r + c:

    OH[rho, 64g+d] = B_[rho, 64g+d] + A[rho, d]
    B_ = v_block @ WVS    WVS = chunk-suffix sums of WV / M (host-precomputed;
                          chunk 15's suffix is zero and not stored)
    A  = TRI^T @ (v_block @ WVR)    WVR = full chunk sum of WV / M

All matmuls in bf16 (tolerance gate is 2e-2; bf16 gives ~3e-3) — 1 cyc/row on
the warm PE (2.4 GHz) vs fp32r's ~1.7, and half the HBM traffic.

wvs column layout per k-tile: [0:512) suffix chunks 0..7, [512:960) suffix
chunks 8..14, [960:1024) WVR.  Per block j: mm1 chain -> psum[j][0:512]
(bank a), mm2 chain -> psum[j][512:1024] (bank b, R at [960:1024)).  PSUM:
4 blocks x 2 banks = all 8 banks; each A matmul reuses its own block's R
region (dead after the ACT rs-copy), so blocks are fully decoupled.

Engine plan per core (4 blocks of 128 rows; 8 cores cover the 32 (b,h)
blocks); in-DMAs are 512KB pairs (the ~1.4us per-transfer fixed cost needs
>=512KB data phases for the two HWDGE rings to reach the ~358GB/s HBM cap):
  gpsimd: memset warm_sb, tri DMA (SWDGE), join on each ring's last out
          transfer, sem range-clear after the exit barrier.
  tensor: 10 warmup dummy matmuls on zeroed SBUF (keep the PE-HAM activity
          window busy at K=4/8 so the clock is at 2.4GHz when real work
          starts), then phase1 = blocks 0,1 interleaved per k-tile with a
          block-2/3 tiles-0,1 filler before the phase1 tail (covers the
          wvs45/wvs67 receipt lag) and block0 frontloaded on the final pair;
          then A0, pass2 (block2 chains, tiles 2..7, A1/A2 woven in at i==5),
          pass3 (block3, A3 at i==5).  A-matmuls sit >=2 matmuls after the
          ACT rs-copies they consume, off the critical path.
  scalar (ACT, ring B HWDGE): in-DMAs; rs_j (psum R -> bf16), a_j
          (a_ps -> a_sb), c15_j (o_sb[960:1024] = A) copies; out1/out3b/out3c.
  vector (DVE): the broadcast adds o_sb = psum + A (addA = chunks 0..7,
          addB = chunks 8..14).  Block 2/3's addB runs while PE is still in
          that block's mm1 chain (different PSUM bank); block3's addA is
          split in two so the final two 128KB out pieces stream down both
          rings in parallel — the tail after the last matmul is only
          addA3 + two small DMAs.
  sync   (ring A HWDGE): in-DMAs, out0/out2/out3a DMAs.
One semaphore per DMA transfer; PE/ACT/DVE event counters per the maps below.
"""

import os
import sys
import types

import numpy as np
import ml_dtypes

if "/opt/trn_rl_repo" not in sys.path:
    sys.path.insert(0, "/opt/trn_rl_repo")

try:
    import antenv.axon_hooks  # noqa: F401
except ImportError:
    _m = types.ModuleType("antenv.axon_hooks")

    def _get_hook():
        try:
            from trn_agent_boot.trn_boot import _ntff_profile_via_ctypes

            return _ntff_profile_via_ctypes("/opt/axon/libaxon_pjrt.so")
        except Exception:
            return None

    _m.get_axon_ntff_profile_hook = _get_hook
    sys.modules["antenv.axon_hooks"] = _m

import concourse.bacc as bacc
import concourse.mybir as mybir
from concourse.bass_utils import run_bass_kernel_spmd

B, S, N = 2, 2048, 1024
H, HD = 16, 64
NB = B * H
N_CORES = 8
PER_CORE = NB // N_CORES  # 4
M_SUM = float(B * H * S * (S - 1) // 2)
K_TILES = 8

F32 = mybir.dt.float32
BF16 = mybir.dt.bfloat16
NP_BF16 = np.dtype(ml_dtypes.bfloat16)

N_DUMMY = 7

_compiled = None
_last_exec_time_ns = None
_last_results = None

# 512KB paired transfers: per-transfer fixed cost (~1.4us, serial per HWDGE
# ring) needs >=512KB data phases for the two rings to hide each other's
# overhead and reach the ~358 GB/s HBM cap.  Ring A (sync) starts issuing
# ~1.5us before ring B (ACT preamble) — wvs01 (first-needed with vt01) on A.
RING_A = ["boot", "wvs23", "wvs45"]
RING_B = ["vt01", "vt23", "wvs67"]
WVS_SEM = {0: "boot", 1: "boot", 2: "wvs23", 3: "wvs23",
           4: "wvs45", 5: "wvs45", 6: "wvs67", 7: "wvs67"}
DMA_NAMES = RING_A + RING_B + ["tri", "o0", "o1", "o2", "o3b", "o3a", "o3c"]
# boot bundle column layout (bf16, [128, 2560]): the first-needed working set
# — vt0 tiles 0,1 | vt1 tiles 0,1 | wvs tiles 0,1 — shipped as ring A's first
# transfer so the PE starts ~2us before the full 512KB pairs land.
BOOT_COLS = 2560

# --- engine event-counter maps (see module docstring) -----------------------
# PE increments, in emission order:
P_J0MM1, P_J0MM2, P_J1MM1, P_J1MM2 = 1, 2, 3, 4
P_A0, P_A1 = 5, 6
P_P2MM2, P_A2, P_P2MM1 = 7, 8, 9
P_P3MM2, P_A3, P_P3MM1 = 10, 11, 12
# ACT increments:
C_RS0, C_RS1, C_A0, C_C150, C_A1, C_C151 = 1, 2, 3, 4, 5, 6
C_RS2, C_A2, C_C152, C_RS3, C_A3, C_C153 = 7, 8, 9, 10, 11, 12
# DVE increments:
V_ADDA0, V_ADDB0, V_ADDA1, V_ADDB1 = 1, 2, 3, 4
V_ADDB2, V_ADDA2, V_ADDB3, V_ADDA3A, V_ADDA3B = 5, 6, 7, 8, 9


def _build_nc():
    nc = bacc.Bacc(
        "TRN2", target_bir_lowering=False, debug=False, enable_asserts=False
    )
    vt_d = nc.dram_tensor("vt", [2, 128, 2, K_TILES, 128], BF16, kind="ExternalInput").ap()
    wvs_d = nc.dram_tensor("wvs", [4, 128, 2, N], BF16, kind="ExternalInput").ap()
    boot_d = nc.dram_tensor("boot", [128, BOOT_COLS], BF16, kind="ExternalInput").ap()
    tri_d = nc.dram_tensor("tri", [128, 128], BF16, kind="ExternalInput").ap()
    out_d = nc.dram_tensor("out", [PER_CORE, 128, N], F32, kind="ExternalOutput").ap()

    wvs_sb = nc.alloc_sbuf_tensor("wvs_sb", [128, K_TILES, N], BF16).ap()
    boot_sb = nc.alloc_sbuf_tensor("boot_sb", [128, BOOT_COLS], BF16).ap()
    tri_sb = nc.alloc_sbuf_tensor("tri_sb", [128, 128], BF16).ap()
    warm_sb = nc.alloc_sbuf_tensor("warm_sb", [128, 512], BF16).ap()
    vtp_sb = [
        nc.alloc_sbuf_tensor(f"vtp_sb{pp}", [128, 2, K_TILES, 128], BF16).ap()
        for pp in range(2)
    ]
    vt_sb = [vtp_sb[j // 2][:, j % 2, :, :] for j in range(PER_CORE)]
    rs_sb = [
        nc.alloc_sbuf_tensor(f"rs_sb{j}", [128, HD], BF16).ap()
        for j in range(PER_CORE)
    ]
    a_sb = [
        nc.alloc_sbuf_tensor(f"a_sb{j}", [128, HD], F32).ap() for j in range(PER_CORE)
    ]
    o_sb = [
        nc.alloc_sbuf_tensor(f"o_sb{j}", [128, N], F32).ap() for j in range(PER_CORE)
    ]

    # 4 x [128,1024] fp32 = 4KB/partition each = 2 PSUM banks each (8 total).
    p = [nc.alloc_psum_tensor(f"p{j}", [128, N], F32).ap() for j in range(PER_CORE)]
    # A-matmul output reuses block j's own R region [960:1024) (bank b),
    # dead once the ACT rs-copy has pulled R out — no cross-block coupling.
    a_ps = [p[j][:, 960:N] for j in range(PER_CORE)]

    sems = {
        k: nc.alloc_semaphore(f"sem_{k}")
        for k in ["warm", "PE", "ACT", "DVE"] + DMA_NAMES
    }
    sem_nums = [s.num for s in sems.values()]
    sem_range = range(min(sem_nums), max(sem_nums) + 1)
    assert max(sem_nums) - min(sem_nums) == len(sem_nums) - 1

    def src(name):
        if name == "boot":
            return boot_d[:]
        if name.startswith("vt"):
            return vt_d[int(name[2]) // 2]
        return wvs_d[int(name[3]) // 2]

    def dst(name):
        if name == "boot":
            return boot_sb[:]
        if name.startswith("vt"):
            return vtp_sb[int(name[2]) // 2][:]
        t0 = int(name[3])
        return wvs_sb[:, t0 : t0 + 2, :]

    # operand views: tiles 0,1 of blocks 0,1 (and wvs tiles 0,1 for every
    # block) come from the boot bundle instead of the paired transfers
    def vt_ap(j, t):
        if j < 2 and t < 2:
            c = 128 * (2 * j + t)
            return boot_sb[:, c : c + 128]
        return vt_sb[j][:, t, :]

    def wvs_ap(t, lo, hi):
        if t < 2:
            c = 512 + 1024 * t
            return boot_sb[:, c + lo : c + hi]
        return wvs_sb[:, t, lo:hi]

    def bc_add(j, lo, hi, psrc_lo, psrc_hi):
        g = (hi - lo) // HD
        return nc.vector.tensor_add(
            o_sb[j][:, lo:hi].rearrange("p (g d) -> p g d", d=HD),
            p[j][:, psrc_lo:psrc_hi].rearrange("p (g d) -> p g d", d=HD),
            a_sb[j][:].unsqueeze(1).broadcast_to([128, g, HD]),
        )

    with nc.Block() as block:

        @block.sync
        def _(sync):
            for name in RING_A:
                sync.dma_start(dst(name), src(name)).then_inc(sems[name], 16)
            sync.wait_ge(sems["DVE"], V_ADDB0)
            sync.wait_ge(sems["ACT"], C_C150)
            sync.dma_start(out_d[0][:], o_sb[0][:]).then_inc(sems["o0"], 16)
            sync.wait_ge(sems["DVE"], V_ADDA2)
            sync.wait_ge(sems["ACT"], C_C152)
            sync.dma_start(out_d[2][:], o_sb[2][:]).then_inc(sems["o2"], 16)
            sync.wait_ge(sems["DVE"], V_ADDA3A)
            sync.dma_start(out_d[3][:, 0:256], o_sb[3][:, 0:256]).then_inc(
                sems["o3a"], 16
            )

        @block.scalar
        def _(scalar):
            for name in RING_B:
                scalar.dma_start(dst(name), src(name)).then_inc(sems[name], 16)

            def rs_copy(j, pe_val):
                scalar.wait_ge(sems["PE"], pe_val)
                nc.scalar.copy(rs_sb[j][:], p[j][:, 960:N]).then_inc(sems["ACT"], 1)

            def a_copy(j, pe_val):
                scalar.wait_ge(sems["PE"], pe_val)
                nc.scalar.copy(a_sb[j][:], a_ps[j]).then_inc(sems["ACT"], 1)

            def c15(j):
                nc.scalar.copy(o_sb[j][:, 960:N], a_sb[j][:]).then_inc(sems["ACT"], 1)

            rs_copy(0, P_J0MM2)
            rs_copy(1, P_J1MM2)
            a_copy(0, P_A0)
            c15(0)
            a_copy(1, P_A1)
            c15(1)
            rs_copy(2, P_P2MM2)
            a_copy(2, P_A2)
            c15(2)
            scalar.wait_ge(sems["DVE"], V_ADDB1)
            scalar.dma_start(out_d[1][:], o_sb[1][:]).then_inc(sems["o1"], 16)
            rs_copy(3, P_P3MM2)
            a_copy(3, P_A3)
            c15(3)
            scalar.wait_ge(sems["DVE"], V_ADDB3)
            scalar.dma_start(out_d[3][:, 512:N], o_sb[3][:, 512:N]).then_inc(
                sems["o3b"], 16
            )
            scalar.wait_ge(sems["DVE"], V_ADDA3B)
            scalar.dma_start(out_d[3][:, 256:512], o_sb[3][:, 256:512]).then_inc(
                sems["o3c"], 16
            )

        @block.tensor
        def _(tensor):
            waited = set()

            def need(name):
                if name in waited:
                    return
                waited.add(name)
                tensor.wait_ge(sems[name], 16)

            def mm(j, t, half, first, last):
                lo = 512 * half
                m = nc.tensor.matmul(
                    p[j][:, lo : lo + 512],
                    vt_ap(j, t),
                    wvs_ap(t, lo, lo + 512),
                    start=first,
                    stop=last,
                    skip_group_check=True,
                )
                if last:
                    m.then_inc(sems["PE"], 1)

            def a_mm(j):
                nc.tensor.matmul(
                    a_ps[j],
                    tri_sb[:],
                    rs_sb[j][:],
                    start=True,
                    stop=True,
                    skip_group_check=True,
                ).then_inc(sems["PE"], 1)

            # warmup: keep the PE busy (HAM window) while the first DMAs land
            tensor.wait_ge(sems["warm"], 1)
            for _i in range(N_DUMMY):
                nc.tensor.matmul(
                    p[2][:, 512:N],
                    warm_sb[:, 0:128],
                    warm_sb[:],
                    start=True,
                    stop=True,
                    skip_group_check=True,
                )

            def spare_dummy(n, cols):
                for _i in range(n):
                    nc.tensor.matmul(
                        p[2][:, 512 : 512 + cols],
                        warm_sb[:, 0:128],
                        warm_sb[:, 0:cols],
                        start=True,
                        stop=True,
                        skip_group_check=True,
                    )

            # phase1: blocks 0,1 interleaved per k-tile, tiles 0..3 (tiles
            # 0,1 read the boot bundle — no vt01/wvs pair needed yet)
            for t in range(4):
                if t == 2:
                    spare_dummy(2, 256)  # cover the wvs23 receipt-lag window
                    need("vt01")
                need(WVS_SEM[t])
                first = t == 0
                for j in (0, 1):
                    mm(j, t, 0, first, False)
                    mm(j, t, 1, first, False)
            # filler while the wvs45/wvs67 receipts land: start blocks 2,3 on
            # the resident tiles 0,1 (must follow the dummy PSUM bank's last
            # spare dummy).
            need("vt23")
            for t in (0, 1):
                first = t == 0
                for j in (2, 3):
                    mm(j, t, 0, first, False)
                    mm(j, t, 1, first, False)
            # phase1 tail: tiles 4..7; block0 frontloaded on the final pair so
            # its combine pipeline starts ~1us early.
            for t in (4, 5):
                need(WVS_SEM[t])
                for j in (0, 1):
                    mm(j, t, 0, False, False)
                    mm(j, t, 1, False, False)
            need("wvs67")
            mm(0, 6, 0, False, False)
            mm(0, 6, 1, False, False)
            mm(0, 7, 0, False, True)  # PE=1
            mm(0, 7, 1, False, True)  # PE=2
            mm(1, 6, 0, False, False)
            mm(1, 6, 1, False, False)
            mm(1, 7, 0, False, True)  # PE=3
            mm(1, 7, 1, False, True)  # PE=4

            tensor.wait_ge(sems["tri"], 16)
            tensor.wait_ge(sems["ACT"], C_RS0)
            a_mm(0)  # PE=5
            # pass2: block 2, mm2 chain then mm1 chain (tiles 2..7; 0,1 were
            # the filler), A-matmuls woven in
            for i in range(2, K_TILES):
                if i == 5:
                    tensor.wait_ge(sems["ACT"], C_RS1)
                    a_mm(1)  # PE=6
                mm(2, i, 1, False, i == K_TILES - 1)  # PE=7
            for i in range(2, K_TILES):
                if i == 5:
                    tensor.wait_ge(sems["ACT"], C_RS2)
                    a_mm(2)  # PE=8
                mm(2, i, 0, False, i == K_TILES - 1)  # PE=9
            # pass3: block 3
            for i in range(2, K_TILES):
                mm(3, i, 1, False, i == K_TILES - 1)  # PE=10
            for i in range(2, K_TILES):
                if i == 5:
                    tensor.wait_ge(sems["ACT"], C_RS3)
                    a_mm(3)  # PE=11
                mm(3, i, 0, False, i == K_TILES - 1)  # PE=12

        @block.vector
        def _(vector):
            def addA(j, act_val=None, pe_val=None, lo=0, hi=512):
                if act_val is not None:
                    vector.wait_ge(sems["ACT"], act_val)
                if pe_val is not None:
                    vector.wait_ge(sems["PE"], pe_val)
                bc_add(j, lo, hi, lo, hi).then_inc(sems["DVE"], 1)

            def addB(j, act_val=None):
                if act_val is not None:
                    vector.wait_ge(sems["ACT"], act_val)
                bc_add(j, 512, 960, 512, 960).then_inc(sems["DVE"], 1)

            addA(0, act_val=C_A0)
            addB(0)
            addA(1, act_val=C_A1)
            addB(1)
            addB(2, act_val=C_A2)  # early: PE still in block2 mm1 chain (bank a)
            addA(2, pe_val=P_P2MM1)
            addB(3, act_val=C_A3)  # early: PE still in block3 mm1 chain
            # last combine split in two so the two final out pieces stream
            # down both rings in parallel
            addA(3, pe_val=P_P3MM1, lo=0, hi=256)    # V_ADDA3A
            addA(3, lo=256, hi=512)                   # V_ADDA3B

        @block.gpsimd
        def _(gpsimd):
            gpsimd.memset(warm_sb[:], 0.0).then_inc(sems["warm"], 1)
            gpsimd.dma_start(tri_sb[:], tri_d[:]).then_inc(sems["tri"], 16)
            # join: each ring's LAST out transfer (HWDGE completion is FIFO
            # per ring; in-DMA sems are all consumed by PE waits already)
            for name in ("o3a", "o3c"):
                gpsimd.wait_ge(sems[name], 16)

    # after the Block's all-engine barrier: restore sems to 0 for reruns
    nc.gpsimd.sem_clear(sem_range)

    nc.compile()
    return nc


def _host_prep(v, WV):
    WVr = WV.astype(np.float64).reshape(N, 16, HD)
    rev = np.flip(np.cumsum(np.flip(WVr, axis=1), axis=1), axis=1)
    WVS = rev - WVr  # exclusive suffix; [:, 15, :] is zero
    WVR = rev[:, 0, :]
    # column layout: [0:512) chunks 0..7, [512:960) chunks 8..14, [960:1024) WVR
    wvs_aug = np.concatenate(
        [WVS[:, :8, :].reshape(N, 512), WVS[:, 8:15, :].reshape(N, 448), WVR],
        axis=1,
    ) / M_SUM
    # pair layout [4, 128, 2, N]: tile-pair p holds tiles 2p, 2p+1
    wvs_aug = np.ascontiguousarray(
        wvs_aug.astype(NP_BF16)
        .reshape(4, 2, 128, N)
        .transpose(0, 2, 1, 3)
    )
    vt_all = np.empty((NB, 128, K_TILES, 128), dtype=NP_BF16)
    for g in range(NB):
        b, h = divmod(g, H)
        vb = v[b, 128 * h : 128 * (h + 1), :]
        vt_all[g] = vb.T.reshape(K_TILES, 128, 128).transpose(1, 0, 2).astype(NP_BF16)
    tri = np.tril(np.ones((128, 128), dtype=np.float32), -1).astype(NP_BF16)
    return vt_all, wvs_aug, tri


def kernel(q, k, v, WQ, WK, WV):
    global _compiled, _last_exec_time_ns, _last_results
    v = np.ascontiguousarray(np.asarray(v, dtype=np.float32))
    WV = np.ascontiguousarray(np.asarray(WV, dtype=np.float32))
    vt_all, wvs_aug, tri = _host_prep(v, WV)

    if _compiled is None:
        _compiled = _build_nc()
    nc = _compiled

    # boot bundle: vt0 tiles 0,1 | vt1 tiles 0,1 | wvs tiles 0,1 (per core)
    wvs01_flat = np.ascontiguousarray(wvs_aug[0].reshape(128, 2 * N))
    in_maps = [
        {
            # block-pair layout [2, 128, 2, K_TILES, 128]
            "vt": np.ascontiguousarray(
                vt_all[PER_CORE * c : PER_CORE * (c + 1)]
                .reshape(2, 2, 128, K_TILES, 128)
                .transpose(0, 2, 1, 3, 4)
            ),
            "wvs": wvs_aug,
            "boot": np.ascontiguousarray(
                np.concatenate(
                    [
                        vt_all[PER_CORE * c][:, 0:2, :].reshape(128, 256),
                        vt_all[PER_CORE * c + 1][:, 0:2, :].reshape(128, 256),
                        wvs01_flat,
                    ],
                    axis=1,
                )
            ),
            "tri": tri,
        }
        for c in range(N_CORES)
    ]
    res = run_bass_kernel_spmd(
        nc,
        in_maps,
        core_ids=list(range(N_CORES)),
        tmpdir=os.environ.get("BASS_KERNEL_TRACE_DIR") or None,
    )
    _last_exec_time_ns = res.exec_time_ns
    _last_results = res

    out = np.empty((B, S, N), dtype=np.float32)
    for c in range(N_CORES):
        oh = res.results[c]["out"]
        for j in range(PER_CORE):
            g = PER_CORE * c + j
            b, h = divmod(g, H)
            out[b, :, HD * h : HD * (h + 1)] = oh[j].reshape(S, HD)
    return out


# revision 59
# speedup vs baseline: 1.1042x; 1.1042x over previous
"""Raw-bass (manual sync) Trainium2 kernel for nn_MultiHeadAttention_79577154060910.

Math (verified vs the jax reference to ~2e-7 rel in fp32, ~3e-3 in bf16): the
reference's GLOBAL softmax (no axis) plus its sign-bugged causal mask
(`S - (1-tril)*(-1e9)` ADDS +1e9 to the strict upper triangle) make the second
softmax's weights an input-independent constant in fp32 arithmetic: every
strictly-upper-triangular position holds exactly 1/M (M = B*H*S*(S-1)/2 =
67076096) and all other positions are exactly exp(-1e9) == 0.  Hence q, k, WQ,
WK never affect the output and

    out[b, q, h*64+d] = (1/M) * sum_{k>q} V[b,h,k,d],  V = (v@WV).reshape(B,H,S,64)

With the raw-reshape head split (V[b,h,k,d] = VV[b, h*128+k//16, (k%16)*64+d]),
each (b,h) maps to a 128-row block of VV and, splitting k = 16r + c:

    OH[rho, 64g+d] = B_[rho, 64g+d] + A[rho, d]
    B_ = v_block @ WVS    WVS = chunk-suffix sums of WV / M (host-precomputed;
                          chunk 15's suffix is zero and not stored)
    A  = TRI^T @ (v_block @ WVR)    WVR = full chunk sum of WV / M

All matmuls in bf16 (tolerance gate is 2e-2; bf16 gives ~3e-3) — 1 cyc/row on
the warm PE (2.4 GHz) vs fp32r's ~1.7, and half the HBM traffic.

wvs column layout per k-tile: [0:512) suffix chunks 0..7, [512:960) suffix
chunks 8..14, [960:1024) WVR.  Per block j: mm1 chain -> psum[j][0:512]
(bank a), mm2 chain -> psum[j][512:1024] (bank b, R at [960:1024)).  PSUM:
4 blocks x 2 banks = all 8 banks; each A matmul reuses its own block's R
region (dead after the ACT rs-copy), so blocks are fully decoupled.

Engine plan per core (4 blocks of 128 rows; 8 cores cover the 32 (b,h)
blocks); in-DMAs are 512KB pairs (the ~1.4us per-transfer fixed cost needs
>=512KB data phases for the two HWDGE rings to reach the ~358GB/s HBM cap):
  gpsimd: memset warm_sb, tri DMA (SWDGE), join on each ring's last out
          transfer, sem range-clear after the exit barrier.
  tensor: 10 warmup dummy matmuls on zeroed SBUF (keep the PE-HAM activity
          window busy at K=4/8 so the clock is at 2.4GHz when real work
          starts), then phase1 = blocks 0,1 interleaved per k-tile with a
          block-2/3 tiles-0,1 filler before the phase1 tail (covers the
          wvs45/wvs67 receipt lag) and block0 frontloaded on the final pair;
          then A0, pass2 (block2 chains, tiles 2..7, A1/A2 woven in at i==5),
          pass3 (block3, A3 at i==5).  A-matmuls sit >=2 matmuls after the
          ACT rs-copies they consume, off the critical path.
  scalar (ACT, ring B HWDGE): in-DMAs; rs_j (psum R -> bf16), a_j
          (a_ps -> a_sb), c15_j (o_sb[960:1024] = A) copies; out1/out3b/out3c.
  vector (DVE): the broadcast adds o_sb = psum + A (addA = chunks 0..7,
          addB = chunks 8..14).  Block 2/3's addB runs while PE is still in
          that block's mm1 chain (different PSUM bank); block3's addA is
          split in two so the final two 128KB out pieces stream down both
          rings in parallel — the tail after the last matmul is only
          addA3 + two small DMAs.
  sync   (ring A HWDGE): in-DMAs, out0/out2/out3a DMAs.
One semaphore per DMA transfer; PE/ACT/DVE event counters per the maps below.
"""

import os
import sys
import types

import numpy as np
import ml_dtypes

if "/opt/trn_rl_repo" not in sys.path:
    sys.path.insert(0, "/opt/trn_rl_repo")

try:
    import antenv.axon_hooks  # noqa: F401
except ImportError:
    _m = types.ModuleType("antenv.axon_hooks")

    def _get_hook():
        try:
            from trn_agent_boot.trn_boot import _ntff_profile_via_ctypes

            return _ntff_profile_via_ctypes("/opt/axon/libaxon_pjrt.so")
        except Exception:
            return None

    _m.get_axon_ntff_profile_hook = _get_hook
    sys.modules["antenv.axon_hooks"] = _m

import concourse.bacc as bacc
import concourse.mybir as mybir
from concourse.bass_utils import run_bass_kernel_spmd

B, S, N = 2, 2048, 1024
H, HD = 16, 64
NB = B * H
N_CORES = 8
PER_CORE = NB // N_CORES  # 4
M_SUM = float(B * H * S * (S - 1) // 2)
K_TILES = 8

F32 = mybir.dt.float32
BF16 = mybir.dt.bfloat16
NP_BF16 = np.dtype(ml_dtypes.bfloat16)

N_DUMMY = 10

_compiled = None
_last_exec_time_ns = None
_last_results = None

# 512KB paired transfers: per-transfer fixed cost (~1.4us, serial per HWDGE
# ring) needs >=512KB data phases for the two rings to hide each other's
# overhead and reach the ~358 GB/s HBM cap.  Ring A (sync) starts issuing
# ~1.5us before ring B (ACT preamble) — wvs01 (first-needed with vt01) on A.
RING_A = ["wvs01", "wvs23", "wvs45"]
RING_B = ["vt01", "vt23", "wvs67"]
WVS_SEM = {0: "wvs01", 1: "wvs01", 2: "wvs23", 3: "wvs23",
           4: "wvs45", 5: "wvs45", 6: "wvs67", 7: "wvs67"}
DMA_NAMES = RING_A + RING_B + ["tri", "o0", "o1", "o2", "o3b", "o3a", "o3c"]

# --- engine event-counter maps (see module docstring) -----------------------
# PE increments, in emission order:
P_J0MM1, P_J0MM2, P_J1MM1, P_J1MM2 = 1, 2, 3, 4
P_A0, P_A1 = 5, 6
P_P2MM2, P_A2, P_P2MM1 = 7, 8, 9
P_P3MM2, P_A3, P_P3MM1 = 10, 11, 12
# ACT increments:
C_RS0, C_RS1, C_A0, C_C150, C_A1, C_C151 = 1, 2, 3, 4, 5, 6
C_RS2, C_A2, C_C152, C_RS3, C_A3, C_C153 = 7, 8, 9, 10, 11, 12
# DVE increments:
V_ADDA0, V_ADDB0, V_ADDA1, V_ADDB1 = 1, 2, 3, 4
V_ADDB2, V_ADDA2, V_ADDB3, V_ADDA3A, V_ADDA3B = 5, 6, 7, 8, 9


def _build_nc():
    nc = bacc.Bacc(
        "TRN2", target_bir_lowering=False, debug=False, enable_asserts=False
    )
    vt_d = nc.dram_tensor("vt", [2, 128, 2, K_TILES, 128], BF16, kind="ExternalInput").ap()
    wvs_d = nc.dram_tensor("wvs", [4, 128, 2, N], BF16, kind="ExternalInput").ap()
    tri_d = nc.dram_tensor("tri", [128, 128], BF16, kind="ExternalInput").ap()
    out_d = nc.dram_tensor("out", [PER_CORE, 128, N], F32, kind="ExternalOutput").ap()

    wvs_sb = nc.alloc_sbuf_tensor("wvs_sb", [128, K_TILES, N], BF16).ap()
    tri_sb = nc.alloc_sbuf_tensor("tri_sb", [128, 128], BF16).ap()
    warm_sb = nc.alloc_sbuf_tensor("warm_sb", [128, 512], BF16).ap()
    vtp_sb = [
        nc.alloc_sbuf_tensor(f"vtp_sb{pp}", [128, 2, K_TILES, 128], BF16).ap()
        for pp in range(2)
    ]
    vt_sb = [vtp_sb[j // 2][:, j % 2, :, :] for j in range(PER_CORE)]
    rs_sb = [
        nc.alloc_sbuf_tensor(f"rs_sb{j}", [128, HD], BF16).ap()
        for j in range(PER_CORE)
    ]
    a_sb = [
        nc.alloc_sbuf_tensor(f"a_sb{j}", [128, HD], F32).ap() for j in range(PER_CORE)
    ]
    o_sb = [
        nc.alloc_sbuf_tensor(f"o_sb{j}", [128, N], F32).ap() for j in range(PER_CORE)
    ]

    # 4 x [128,1024] fp32 = 4KB/partition each = 2 PSUM banks each (8 total).
    p = [nc.alloc_psum_tensor(f"p{j}", [128, N], F32).ap() for j in range(PER_CORE)]
    # A-matmul output reuses block j's own R region [960:1024) (bank b),
    # dead once the ACT rs-copy has pulled R out — no cross-block coupling.
    a_ps = [p[j][:, 960:N] for j in range(PER_CORE)]

    sems = {
        k: nc.alloc_semaphore(f"sem_{k}")
        for k in ["warm", "PE", "ACT", "DVE"] + DMA_NAMES
    }
    sem_nums = [s.num for s in sems.values()]
    sem_range = range(min(sem_nums), max(sem_nums) + 1)
    assert max(sem_nums) - min(sem_nums) == len(sem_nums) - 1

    def src(name):
        if name.startswith("vt"):
            return vt_d[int(name[2]) // 2]
        return wvs_d[int(name[3]) // 2]

    def dst(name):
        if name.startswith("vt"):
            return vtp_sb[int(name[2]) // 2][:]
        t0 = int(name[3])
        return wvs_sb[:, t0 : t0 + 2, :]

    def bc_add(j, lo, hi, psrc_lo, psrc_hi):
        g = (hi - lo) // HD
        return nc.vector.tensor_add(
            o_sb[j][:, lo:hi].rearrange("p (g d) -> p g d", d=HD),
            p[j][:, psrc_lo:psrc_hi].rearrange("p (g d) -> p g d", d=HD),
            a_sb[j][:].unsqueeze(1).broadcast_to([128, g, HD]),
        )

    with nc.Block() as block:

        @block.sync
        def _(sync):
            for name in RING_A:
                sync.dma_start(dst(name), src(name)).then_inc(sems[name], 16)
            sync.wait_ge(sems["DVE"], V_ADDB0)
            sync.wait_ge(sems["ACT"], C_C150)
            sync.dma_start(out_d[0][:], o_sb[0][:]).then_inc(sems["o0"], 16)
            sync.wait_ge(sems["DVE"], V_ADDA2)
            sync.wait_ge(sems["ACT"], C_C152)
            sync.dma_start(out_d[2][:], o_sb[2][:]).then_inc(sems["o2"], 16)
            sync.wait_ge(sems["DVE"], V_ADDA3A)
            sync.dma_start(out_d[3][:, 0:256], o_sb[3][:, 0:256]).then_inc(
                sems["o3a"], 16
            )

        @block.scalar
        def _(scalar):
            for name in RING_B:
                scalar.dma_start(dst(name), src(name)).then_inc(sems[name], 16)

            def rs_copy(j, pe_val):
                scalar.wait_ge(sems["PE"], pe_val)
                nc.scalar.copy(rs_sb[j][:], p[j][:, 960:N]).then_inc(sems["ACT"], 1)

            def a_copy(j, pe_val):
                scalar.wait_ge(sems["PE"], pe_val)
                nc.scalar.copy(a_sb[j][:], a_ps[j]).then_inc(sems["ACT"], 1)

            def c15(j):
                nc.scalar.copy(o_sb[j][:, 960:N], a_sb[j][:]).then_inc(sems["ACT"], 1)

            rs_copy(0, P_J0MM2)
            rs_copy(1, P_J1MM2)
            a_copy(0, P_A0)
            c15(0)
            a_copy(1, P_A1)
            c15(1)
            rs_copy(2, P_P2MM2)
            a_copy(2, P_A2)
            c15(2)
            scalar.wait_ge(sems["DVE"], V_ADDB1)
            scalar.dma_start(out_d[1][:], o_sb[1][:]).then_inc(sems["o1"], 16)
            rs_copy(3, P_P3MM2)
            a_copy(3, P_A3)
            c15(3)
            scalar.wait_ge(sems["DVE"], V_ADDB3)
            scalar.dma_start(out_d[3][:, 512:N], o_sb[3][:, 512:N]).then_inc(
                sems["o3b"], 16
            )
            scalar.wait_ge(sems["DVE"], V_ADDA3B)
            scalar.dma_start(out_d[3][:, 256:512], o_sb[3][:, 256:512]).then_inc(
                sems["o3c"], 16
            )

        @block.tensor
        def _(tensor):
            waited = set()

            def need(name):
                if name in waited:
                    return
                waited.add(name)
                tensor.wait_ge(sems[name], 16)

            def mm(j, t, half, first, last):
                lo = 512 * half
                m = nc.tensor.matmul(
                    p[j][:, lo : lo + 512],
                    vt_sb[j][:, t, :],
                    wvs_sb[:, t, lo : lo + 512],
                    start=first,
                    stop=last,
                    skip_group_check=True,
                )
                if last:
                    m.then_inc(sems["PE"], 1)

            def a_mm(j):
                nc.tensor.matmul(
                    a_ps[j],
                    tri_sb[:],
                    rs_sb[j][:],
                    start=True,
                    stop=True,
                    skip_group_check=True,
                ).then_inc(sems["PE"], 1)

            # warmup: keep the PE busy (HAM window) while the first DMAs land
            tensor.wait_ge(sems["warm"], 1)
            for _i in range(N_DUMMY):
                nc.tensor.matmul(
                    p[2][:, 512:N],
                    warm_sb[:, 0:128],
                    warm_sb[:],
                    start=True,
                    stop=True,
                    skip_group_check=True,
                )

            def spare_dummy(n, cols):
                for _i in range(n):
                    nc.tensor.matmul(
                        p[2][:, 512 : 512 + cols],
                        warm_sb[:, 0:128],
                        warm_sb[:, 0:cols],
                        start=True,
                        stop=True,
                        skip_group_check=True,
                    )

            # phase1: blocks 0,1 interleaved per k-tile, tiles 0..3
            for t in range(4):
                if t == 2:
                    spare_dummy(2, 256)  # cover the wvs23 receipt-lag window
                need(WVS_SEM[t])
                need("vt01")
                first = t == 0
                for j in (0, 1):
                    mm(j, t, 0, first, False)
                    mm(j, t, 1, first, False)
            # filler while the wvs45/wvs67 receipts land: start blocks 2,3 on
            # the resident tiles 0,1 (must follow the dummy PSUM bank's last
            # spare dummy).
            need("vt23")
            for t in (0, 1):
                first = t == 0
                for j in (2, 3):
                    mm(j, t, 0, first, False)
                    mm(j, t, 1, first, False)
            # phase1 tail: tiles 4..7; block0 frontloaded on the final pair so
            # its combine pipeline starts ~1us early.
            for t in (4, 5):
                need(WVS_SEM[t])
                for j in (0, 1):
                    mm(j, t, 0, False, False)
                    mm(j, t, 1, False, False)
            need("wvs67")
            mm(0, 6, 0, False, False)
            mm(0, 6, 1, False, False)
            mm(0, 7, 0, False, True)  # PE=1
            mm(0, 7, 1, False, True)  # PE=2
            mm(1, 6, 0, False, False)
            mm(1, 6, 1, False, False)
            mm(1, 7, 0, False, True)  # PE=3
            mm(1, 7, 1, False, True)  # PE=4

            tensor.wait_ge(sems["tri"], 16)
            tensor.wait_ge(sems["ACT"], C_RS0)
            a_mm(0)  # PE=5
            # pass2: block 2, mm2 chain then mm1 chain (tiles 2..7; 0,1 were
            # the filler), A-matmuls woven in
            for i in range(2, K_TILES):
                if i == 5:
                    tensor.wait_ge(sems["ACT"], C_RS1)
                    a_mm(1)  # PE=6
                mm(2, i, 1, False, i == K_TILES - 1)  # PE=7
            for i in range(2, K_TILES):
                if i == 5:
                    tensor.wait_ge(sems["ACT"], C_RS2)
                    a_mm(2)  # PE=8
                mm(2, i, 0, False, i == K_TILES - 1)  # PE=9
            # pass3: block 3
            for i in range(2, K_TILES):
                mm(3, i, 1, False, i == K_TILES - 1)  # PE=10
            for i in range(2, K_TILES):
                if i == 5:
                    tensor.wait_ge(sems["ACT"], C_RS3)
                    a_mm(3)  # PE=11
                mm(3, i, 0, False, i == K_TILES - 1)  # PE=12

        @block.vector
        def _(vector):
            def addA(j, act_val=None, pe_val=None, lo=0, hi=512):
                if act_val is not None:
                    vector.wait_ge(sems["ACT"], act_val)
                if pe_val is not None:
                    vector.wait_ge(sems["PE"], pe_val)
                bc_add(j, lo, hi, lo, hi).then_inc(sems["DVE"], 1)

            def addB(j, act_val=None):
                if act_val is not None:
                    vector.wait_ge(sems["ACT"], act_val)
                bc_add(j, 512, 960, 512, 960).then_inc(sems["DVE"], 1)

            addA(0, act_val=C_A0)
            addB(0)
            addA(1, act_val=C_A1)
            addB(1)
            addB(2, act_val=C_A2)  # early: PE still in block2 mm1 chain (bank a)
            addA(2, pe_val=P_P2MM1)
            addB(3, act_val=C_A3)  # early: PE still in block3 mm1 chain
            # last combine split in two so the two final out pieces stream
            # down both rings in parallel
            addA(3, pe_val=P_P3MM1, lo=0, hi=256)    # V_ADDA3A
            addA(3, lo=256, hi=512)                   # V_ADDA3B

        @block.gpsimd
        def _(gpsimd):
            gpsimd.memset(warm_sb[:], 0.0).then_inc(sems["warm"], 1)
            gpsimd.dma_start(tri_sb[:], tri_d[:]).then_inc(sems["tri"], 16)
            # join: each ring's LAST out transfer (HWDGE completion is FIFO
            # per ring; in-DMA sems are all consumed by PE waits already)
            for name in ("o3a", "o3c"):
                gpsimd.wait_ge(sems[name], 16)

    # after the Block's all-engine barrier: restore sems to 0 for reruns
    nc.gpsimd.sem_clear(sem_range)

    nc.compile()
    return nc


def _host_prep(v, WV):
    WVr = WV.astype(np.float64).reshape(N, 16, HD)
    rev = np.flip(np.cumsum(np.flip(WVr, axis=1), axis=1), axis=1)
    WVS = rev - WVr  # exclusive suffix; [:, 15, :] is zero
    WVR = rev[:, 0, :]
    # column layout: [0:512) chunks 0..7, [512:960) chunks 8..14, [960:1024) WVR
    wvs_aug = np.concatenate(
        [WVS[:, :8, :].reshape(N, 512), WVS[:, 8:15, :].reshape(N, 448), WVR],
        axis=1,
    ) / M_SUM
    # pair layout [4, 128, 2, N]: tile-pair p holds tiles 2p, 2p+1
    wvs_aug = np.ascontiguousarray(
        wvs_aug.astype(NP_BF16)
        .reshape(4, 2, 128, N)
        .transpose(0, 2, 1, 3)
    )
    vt_all = np.empty((NB, 128, K_TILES, 128), dtype=NP_BF16)
    for g in range(NB):
        b, h = divmod(g, H)
        vb = v[b, 128 * h : 128 * (h + 1), :]
        vt_all[g] = vb.T.reshape(K_TILES, 128, 128).transpose(1, 0, 2).astype(NP_BF16)
    tri = np.tril(np.ones((128, 128), dtype=np.float32), -1).astype(NP_BF16)
    return vt_all, wvs_aug, tri


def kernel(q, k, v, WQ, WK, WV):
    global _compiled, _last_exec_time_ns, _last_results
    v = np.ascontiguousarray(np.asarray(v, dtype=np.float32))
    WV = np.ascontiguousarray(np.asarray(WV, dtype=np.float32))
    vt_all, wvs_aug, tri = _host_prep(v, WV)

    if _compiled is None:
        _compiled = _build_nc()
    nc = _compiled

    in_maps = [
        {
            # block-pair layout [2, 128, 2, K_TILES, 128]
            "vt": np.ascontiguousarray(
                vt_all[PER_CORE * c : PER_CORE * (c + 1)]
                .reshape(2, 2, 128, K_TILES, 128)
                .transpose(0, 2, 1, 3, 4)
            ),
            "wvs": wvs_aug,
            "tri": tri,
        }
        for c in range(N_CORES)
    ]
    res = run_bass_kernel_spmd(
        nc,
        in_maps,
        core_ids=list(range(N_CORES)),
        tmpdir=os.environ.get("BASS_KERNEL_TRACE_DIR") or None,
    )
    _last_exec_time_ns = res.exec_time_ns
    _last_results = res

    out = np.empty((B, S, N), dtype=np.float32)
    for c in range(N_CORES):
        oh = res.results[c]["out"]
        for j in range(PER_CORE):
            g = PER_CORE * c + j
            b, h = divmod(g, H)
            out[b, :, HD * h : HD * (h + 1)] = oh[j].reshape(S, HD)
    return out
